# revision 40
# baseline (speedup 1.0000x reference)
"""Trainium2 Bass kernel for nn_EvolvingSystem (moe_routing).

Math (reference):
  psi = softmax_c(-d2),  d2[b,c] = (mu_c - z_b)^T S_c (mu_c - z_b),  S_c = si_c si_c^T
  ARX: preds[b,c,l] from linear recursion on state0 = y[:,:,-16:] and
       ub[b,c] = u[b,c,:].b_coef[c] + bias[c]
  out[b,l] = sum_c psi[b,c] preds[b,c,l]

Device strategy (8 cores, data-parallel on B, 1024 rows/core):
  d2[b,c] = ||t_bc||^2 - 2 z_b.q_c + k_c   with t_bc = si_c^T z_b
    -> big matmul T = Z @ si_c per cluster pair (fp32r, full PE rate),
       z^T tiles stationary (lhsT), sigma streams (rhs, N=512).
  Cluster columns are INTERLEAVED in each [128,512] tile (even/odd lanes =
  the two clusters). Sum-of-squares drains split per (pair,bk) tile across
  the two PSUM-capable vector engines (GPSIMD/Pool cannot touch PSUM, and
  DVE may read only one PSUM operand, which rules out tensor_tensor_reduce):
    T-tiles (44): DVE bn_stats (sumsq = M2 + 256*mean^2 fixup per bk)
    A-tiles (20): ACT Square+accum_out per cluster on strided lanes
  The per-bk T-pair sets are kept to all/evens/odds so the fixup writes are
  regular rearrange views. Softmax keeps the min-subtraction pass: tr(S) is
  ~90 so d2 reaches ~130 and raw exp(-d2) underflows f32 for ~12% of rows.
  ARX recursion is linear -> host-unrolled coefficients W[c,l,o], g[c,l]:
    preds[b,c,l] = sum_o W[c,l,o] state0[b,c,o] + g[c,l] ub[b,c]
    out^T[l,b] = Wflat^T @ (psi*state0)^T + g^T @ (psi*ub)^T   (small matmuls)
  u/state0/bmat ship as bf16 (error contribution measured ~0); z/sigma must
  stay fp32 (bf16 sigma alone puts final rel-err over the 2e-2 gate).

DMA: few, large transfers. sync(SP) queue: z khalf0 + 8 sigma slabs
[128,1024] + s0t + ut; scalar(ACT) queue: z khalf1 + packed params (its
activation work starts only after pair0 lands). Pool only computes.
Softmax for batch-chunk bk is emitted right after the last pair's drain of
that bk so it overlaps the remaining main matmuls.
"""

import sys
from contextlib import ExitStack

import numpy as np

if "/opt/trn_rl_repo" not in sys.path:
    sys.path.insert(0, "/opt/trn_rl_repo")

import ml_dtypes

import concourse.bass as bass
import concourse.mybir as mybir
import concourse.tile as tile
from concourse import bacc
from concourse.bass_utils import run_bass_kernel_spmd

N_CORES = 8
B, C, D = 8192, 16, 256
R, E, ORD, L = 64, 32, 16, 32
BLOC = B // N_CORES            # 1024
NBK = BLOC // 128              # 8 batch chunks of 128
CE = C * E                     # 512
CO = C * ORD                   # 256
NPAIR = C // 2                 # 8 cluster pairs
NSLAB = NPAIR // 2             # 4 sigma DMA slabs (2 pairs each)

F32 = mybir.dt.float32
F32R = mybir.dt.float32r
BF16 = mybir.dt.bfloat16
F16 = mybir.dt.float16

# per-(pair,bk) drain route: T=DVE bn_stats, A=ACT square+accum.
# 44 T / 20 A balances DVE vs ACT busy time; each bk's T-pair set is
# all/evens/odds so the fixup output views stay regular.
_ALLB, _EVENB, _ODDB = {1, 4, 6}, {0, 3, 5}, {2, 7}


def _is_t(pair, bk):
    return (
        bk in _ALLB
        or (bk in _EVENB and pair % 2 == 0)
        or (bk in _ODDB and pair % 2 == 1)
    )


TSET = [
    tuple(p for p in range(NPAIR) if _is_t(p, bk)) for bk in range(NBK)
]

_CACHE = {}


def build_program():
    nc = bacc.Bacc(
        "TRN2",
        target_bir_lowering=False,
        debug=False,
        enable_asserts=False,
        num_devices=N_CORES,
    )

    # ---- DRAM I/O (per-core shapes) ----
    zta_d = nc.dram_tensor("zta", [128, BLOC], F32R, kind="ExternalInput").ap()
    ztb_d = nc.dram_tensor("ztb", [128, BLOC], F32R, kind="ExternalInput").ap()
    # qa[i, 2*half+c] = -2 q[c, half*128+i];  kfill[i, bk*16+c] = k_c
    qa_d = nc.dram_tensor("qa", [128, 2 * C], F32R, kind="ExternalInput").ap()
    krow_d = nc.dram_tensor("krow", [1, 128], F32R, kind="ExternalInput").ap()
    # sgr[h, i, c*256 + j] = sigma_inv[c, h*128+i, j]  (cluster-major cols)
    sgr_d = nc.dram_tensor("sgr", [2, 128, NPAIR * 512], F32R, kind="ExternalInput").ap()
    s0t_d = nc.dram_tensor("s0t", [128, 2 * BLOC], BF16, kind="ExternalInput").ap()
    ut_d = nc.dram_tensor("ut", [128, 4 * BLOC], BF16, kind="ExternalInput").ap()
    # pk16 = [emat | gmat | biasv] on 16 partitions
    pk16_d = nc.dram_tensor("pk16", [C, CO + L + 1], F32R, kind="ExternalInput").ap()
    wfp_d = nc.dram_tensor("wfp", [128, 2 * L], F32R, kind="ExternalInput").ap()
    bmp_d = nc.dram_tensor("bmp", [128, 4 * C], BF16, kind="ExternalInput").ap()
    ident_d = nc.dram_tensor("ident", [128, 128], F32, kind="ExternalInput").ap()
    out_d = nc.dram_tensor("outT", [L, BLOC], F32, kind="ExternalOutput").ap()

    with tile.TileContext(nc) as tc, ExitStack() as ctx:
        const = ctx.enter_context(tc.tile_pool(name="const", bufs=1))
        scr_a = ctx.enter_context(tc.tile_pool(name="scr_a", bufs=3))
        sqp = ctx.enter_context(tc.tile_pool(name="sqp", bufs=NBK))
        stp = ctx.enter_context(tc.tile_pool(name="stp", bufs=NBK))
        soft = ctx.enter_context(tc.tile_pool(name="soft", bufs=4))
        tailp = ctx.enter_context(tc.tile_pool(name="tailp", bufs=4))
        ps_t = ctx.enter_context(tc.tile_pool(name="ps_t", bufs=5, space="PSUM"))
        ps_dots = ctx.enter_context(tc.tile_pool(name="ps_dots", bufs=1, space="PSUM"))
        ps_tail = ctx.enter_context(tc.tile_pool(name="ps_tail", bufs=2, space="PSUM"))

        # ---- startup loads ----
        # A single DMA's transfer streams at ~41 B/ns (~2 of the 16 rings)
        # with ~3 in flight per queue, so large tensors are CHUNKED and
        # issued in need-order across the queues. s0t/ut trail the sigma
        # stream so they don't steal bandwidth from the critical path.
        zta = const.tile([128, BLOC], F32R, tag="zta", name="zta")
        ztb = const.tile([128, BLOC], F32R, tag="ztb", name="ztb")
        sg = [
            [const.tile([128, 512], F32R, tag=f"sg{p}{h}", name=f"sg{p}{h}")
             for h in range(2)]
            for p in range(NPAIR)
        ]
        qa = const.tile([128, 2 * C], F32R, tag="qa", name="qa")
        dots = ps_dots.tile([128, 128], F32, tag="dots", name="dots")

        def zc(i):
            return slice(i * 256, (i + 1) * 256)

        # sync: zta chunks + sigma khalf0 + tail tensors
        nc.sync.dma_start(zta[:, zc(0)], zta_d[:, zc(0)])
        nc.sync.dma_start(sg[0][0][:], sgr_d[0, :, 0:512])
        for i in range(1, 4):
            nc.sync.dma_start(zta[:, zc(i)], zta_d[:, zc(i)])
        for p in range(1, NPAIR):
            nc.sync.dma_start(sg[p][0][:], sgr_d[0, :, p * 512:(p + 1) * 512])
        s0t = const.tile([128, 2 * BLOC], BF16, tag="s0t", name="s0t")
        ut = const.tile([128, 4 * BLOC], BF16, tag="ut", name="ut")
        for i in range(2):
            cs = slice(i * BLOC, (i + 1) * BLOC)
            nc.sync.dma_start(s0t[:, cs], s0t_d[:, cs])
        for i in range(4):
            cs = slice(i * BLOC, (i + 1) * BLOC)
            nc.sync.dma_start(ut[:, cs], ut_d[:, cs])

        # scalar: qa/kfill/ztb chunks + sigma khalf1 (done before ACT compute)
        nc.scalar.dma_start(qa[:], qa_d[:])
        nc.scalar.dma_start(ztb[:, zc(0)], ztb_d[:, zc(0)])
        nc.scalar.dma_start(sg[0][1][:], sgr_d[1, :, 0:512])
        krow = const.tile([1, 128], F32R, tag="krow", name="krow")
        nc.scalar.dma_start(krow[:], krow_d[:])
        for i in range(1, 4):
            nc.scalar.dma_start(ztb[:, zc(i)], ztb_d[:, zc(i)])
        for p in range(1, NPAIR):
            nc.scalar.dma_start(sg[p][1][:], sgr_d[1, :, p * 512:(p + 1) * 512])

        # gpsimd: small params via SWDGE
        pk16 = const.tile([C, CO + L + 1], F32R, tag="pk16", name="pk16")
        nc.gpsimd.dma_start(pk16[:], pk16_d[:])
        wfp = const.tile([128, 2 * L], F32R, tag="wfp", name="wfp")
        nc.gpsimd.dma_start(wfp[:], wfp_d[:])
        bmp = const.tile([128, 4 * C], BF16, tag="bmp", name="bmp")
        nc.gpsimd.dma_start(bmp[:], bmp_d[:])
        ident = const.tile([128, 128], F32, tag="ident", name="ident")
        nc.gpsimd.dma_start(ident[:], ident_d[:])

        emat = pk16[:, 0:CO]
        gmat = pk16[:, CO:CO + L]
        biasv = pk16[:, CO + L:CO + L + 1]

        # ---- dots[b, bk*16+c] = -2 z.q + k  (one PSUM bank, col-sliced) ----
        # One rank-1 matmul broadcasts k into the whole bank (ones^T @ krow),
        # then the per-bk slices accumulate on top - no per-bk ones matmuls.
        ones = const.tile([1, 128], F32R, tag="ones", name="ones")
        nc.gpsimd.memset(ones[:].bitcast(mybir.dt.uint32), 0x3F800000)
        nc.tensor.matmul(dots[:], ones[:], krow[:], start=True, stop=False,
                         skip_group_check=True)
        for bk in range(NBK):
            sl = dots[:, bk * C:(bk + 1) * C]
            bsl = slice(bk * 128, (bk + 1) * 128)
            nc.tensor.matmul(sl, zta[:, bsl], qa[:, 0:C], start=False,
                             stop=False, skip_group_check=True)
            nc.tensor.matmul(sl, ztb[:, bsl], qa[:, C:2 * C], start=False,
                             stop=True, skip_group_check=True)

        sqacc = [sqp.tile([128, C], F32, tag="sqacc", name="sqacc") for _ in range(NBK)]
        stats = [
            stp.tile([128, NPAIR, 6], F32, tag="stats", name="stats")
            for _ in range(NBK)
        ]
        psit_r = const.tile([C, BLOC], F32R, tag="psit_r", name="psit_r")
        psi4 = [
            const.tile([128, 128], F32, tag=f"psi4{g}", name=f"psi4{g}")
            for g in range(2)
        ]
        for g in range(2):
            # pad lanes are transposed and then ignored; zero them so the
            # simulator never sees uninitialized reads
            nc.gpsimd.memset(psi4[g][:].bitcast(mybir.dt.uint32), 0)

        def drain(pair, bk, t_ps, eng):
            if eng == "T":
                slot = TSET[bk].index(pair)
                nc.vector.bn_stats(stats[bk][:, slot, :], t_ps[:])
            else:
                for cc in range(2):
                    acc = sqacc[bk][:, 2 * pair + cc:2 * pair + cc + 1]
                    o = scr_a.tile([128, 256], F32, tag="scra", name="scra")
                    nc.scalar.activation(
                        o[:], t_ps[:, cc::2],
                        mybir.ActivationFunctionType.Square,
                        accum_out=acc,
                    )

        def fixup_bk(bk):
            # DVE tiles: sumsq = M2 + 256*mean^2 per cluster (even/odd stats)
            ts = TSET[bk]
            if not ts:
                return
            nt = len(ts)
            st = stats[bk]
            v_mu = st[:, 0:nt, 1:6:3]    # [128, nt, 2] means (even, odd)
            v_m2 = st[:, 0:nt, 2:6:3]    # [128, nt, 2] M2 = count*var
            if nt == NPAIR:
                o = sqacc[bk][:, 0:16].rearrange("p (g x) -> p g x", x=2)
            elif ts[0] == 0:             # even pairs -> cols (4g, 4g+1)
                o = sqacc[bk][:, 0:16].rearrange("p (g x) -> p g x", x=4)[:, :, 0:2]
            else:                        # odd pairs -> cols (4g+2, 4g+3)
                o = sqacc[bk][:, 0:16].rearrange("p (g x) -> p g x", x=4)[:, :, 2:4]
            tmp = soft.tile([128, nt, 2], F32, tag="fix", name="fix")
            nc.vector.tensor_tensor(tmp[:], v_mu, v_mu, op=mybir.AluOpType.mult)
            nc.vector.scalar_tensor_tensor(
                out=o, in0=tmp[:], scalar=256.0, in1=v_m2,
                op0=mybir.AluOpType.mult, op1=mybir.AluOpType.add,
            )

        def softmax_bk(bk):
            fixup_bk(bk)
            d2 = soft.tile([128, C], F32, tag="d2", name="d2")
            nc.vector.scalar_tensor_tensor(
                out=d2[:], in0=dots[:, bk * C:(bk + 1) * C], scalar=1.0,
                in1=sqacc[bk][:], op0=mybir.AluOpType.mult,
                op1=mybir.AluOpType.add,
            )
            dmin = soft.tile([128, 1], F32, tag="dmin", name="dmin")
            nc.vector.tensor_reduce(
                dmin[:], d2[:], axis=mybir.AxisListType.X, op=mybir.AluOpType.min
            )
            et = soft.tile([128, C], F32, tag="et", name="et")
            den = soft.tile([128, 1], F32, tag="den", name="den")
            nc.scalar.activation(
                et[:], d2[:], mybir.ActivationFunctionType.Exp,
                bias=dmin[:], scale=-1.0, accum_out=den[:],
            )
            rden = soft.tile([128, 1], F32, tag="rden", name="rden")
            nc.vector.reciprocal(rden[:], den[:])
            g = bk // 4
            # each chunk's psi sits at a 32-col boundary so the post-transpose
            # partition offsets (32j) are engine-legal
            nc.vector.tensor_scalar_mul(
                psi4[g][:, (bk % 4) * 32:(bk % 4) * 32 + C], et[:], rden[:]
            )
            if bk % 4 == 3:
                # one PE transpose per 4 batch chunks, then per-chunk copies
                pt_ps = ps_tail.tile([128, 128], F32, tag="tail", name="tail")
                nc.tensor.transpose(pt_ps[:], psi4[g][:], ident[:])
                for j in range(4):
                    nc.scalar.activation(
                        psit_r[:, (4 * g + j) * 128:(4 * g + j + 1) * 128],
                        pt_ps[j * 32:j * 32 + C, :],
                        mybir.ActivationFunctionType.Copy,
                    )

        # ---- main: T = Z @ si per (pair, bk); rotating 3-engine drain ----
        for pair in range(NPAIR):
            sg0 = sg[pair][0][:]
            sg1 = sg[pair][1][:]
            for bk in range(NBK):
                bsl = slice(bk * 128, (bk + 1) * 128)
                t_ps = ps_t.tile([128, 512], F32, tag="t_ps", name="t_ps")
                nc.tensor.matmul(t_ps[:], zta[:, bsl], sg0, start=True, stop=False)
                nc.tensor.matmul(t_ps[:], ztb[:, bsl], sg1, start=False, stop=True)
                drain(pair, bk, t_ps, "T" if _is_t(pair, bk) else "A")
                if pair == NPAIR - 1:
                    softmax_bk(bk)

        # ---- tail (all in [*, b] orientation, b512 chunks) ----
        for bh in range(2):
            bsl = slice(bh * 512, (bh + 1) * 512)
            psie = []
            for k in range(2):
                p = ps_tail.tile([128, 512], F32, tag="tail", name="tail")
                nc.tensor.matmul(
                    p[:], emat[:, k * 128:(k + 1) * 128], psit_r[:, bsl],
                    start=True, stop=True,
                )
                psie.append(p)
            a_sb = []
            for k in range(2):
                t = tailp.tile([128, 512], F32R, tag="a_sb", name="a_sb")
                nc.vector.tensor_tensor(
                    t[:], s0t[:, k * BLOC + bh * 512:k * BLOC + bh * 512 + 512],
                    psie[k][:], op=mybir.AluOpType.mult,
                )
                a_sb.append(t)
            ubp = ps_tail.tile([C, 512], F32, tag="tail", name="tail")
            for k in range(4):
                nc.tensor.matmul(
                    ubp[:], bmp[:, k * C:(k + 1) * C],
                    ut[:, k * BLOC + bh * 512:k * BLOC + bh * 512 + 512],
                    start=(k == 0), stop=(k == 3),
                )
            pt_sb = tailp.tile([C, 512], F32R, tag="pt_sb", name="pt_sb")
            nc.vector.scalar_tensor_tensor(
                out=pt_sb[:], in0=ubp[:], scalar=biasv, in1=psit_r[:, bsl],
                op0=mybir.AluOpType.add, op1=mybir.AluOpType.mult,
            )
            outp = ps_tail.tile([L, 512], F32, tag="tail", name="tail")
            nc.tensor.matmul(outp[:], wfp[:, 0:L], a_sb[0][:], start=True, stop=False)
            nc.tensor.matmul(outp[:], wfp[:, L:2 * L], a_sb[1][:], start=False, stop=False)
            nc.tensor.matmul(outp[:], gmat, pt_sb[:], start=False, stop=True)
            out_sb = tailp.tile([L, 512], F32, tag="out_sb", name="out_sb")
            nc.vector.tensor_copy(out_sb[:], outp[:])
            nc.sync.dma_start(out_d[:, bsl], out_sb[:])

    nc.compile()
    return nc


def host_prep(y, z, u, mu, sigma_inv, a_coef, b_coef, bias):
    """Host-side precompute: shared tensors + per-core input maps."""
    f64 = np.float64
    bf = ml_dtypes.bfloat16
    W = np.zeros((C, L, ORD), f64)
    g = np.zeros((C, L), f64)
    for c in range(C):
        a = a_coef[c].astype(f64)
        S = np.eye(ORD, dtype=f64)
        sb = np.zeros(ORD, f64)
        for l in range(L):
            ya = a @ S
            yb = a @ sb + 1.0
            W[c, l] = ya
            g[c, l] = yb
            S = np.vstack([S[1:], ya[None]])
            sb = np.concatenate([sb[1:], [yb]])
    wflat = np.ascontiguousarray(W.transpose(0, 2, 1).reshape(CO, L)).astype(np.float32)
    wfp = np.concatenate([wflat[0:128], wflat[128:256]], axis=1)
    gmat = g.astype(np.float32)

    si = sigma_inv.astype(f64)
    m = np.einsum("cij,ci->cj", si, mu.astype(f64))
    q = np.einsum("cij,cj->ci", si, m)          # S_c mu_c
    k = np.sum(m * m, axis=1)
    qt = (-2.0 * q.T).astype(np.float32)        # [D, C]
    qa = np.concatenate([qt[0:128], qt[128:256]], axis=1)   # [128, 2C]
    krow = np.tile(k.astype(np.float32), NBK).reshape(1, 128)

    # interleave each pair's two clusters in the column lanes (even/odd)
    sit = sigma_inv.transpose(1, 0, 2)          # [i, c, j]
    sgr = np.ascontiguousarray(
        sit.reshape(D, NPAIR, 2, D).transpose(0, 1, 3, 2).reshape(2, 128, NPAIR * 512)
    ).astype(np.float32)

    emat = np.zeros((C, CO), np.float32)
    for c in range(C):
        emat[c, c * ORD:(c + 1) * ORD] = 1.0
    pk16 = np.concatenate(
        [emat, gmat, bias.astype(np.float32).reshape(C, 1)], axis=1
    )

    bmat = np.zeros((CE, C), np.float32)
    for c in range(C):
        bmat[c * E:(c + 1) * E, c] = b_coef[c]
    bmp = np.concatenate([bmat[k * 128:(k + 1) * 128] for k in range(4)], axis=1)

    shared = {
        "qa": qa,
        "krow": krow,
        "sgr": sgr,
        "pk16": pk16,
        "wfp": wfp,
        "bmp": bmp.astype(bf),
        "ident": np.eye(128, dtype=np.float32),
    }
    in_maps = []
    for i in range(N_CORES):
        s = slice(i * BLOC, (i + 1) * BLOC)
        zt = np.ascontiguousarray(z[s, 0, :].T)             # [256, BLOC]
        s0 = np.ascontiguousarray(y[s, :, R - ORD:].reshape(BLOC, CO).T)
        utt = np.ascontiguousarray(u[s].reshape(BLOC, CE).T)
        m_i = dict(shared)
        m_i["zta"] = zt[0:128]
        m_i["ztb"] = zt[128:256]
        m_i["s0t"] = np.concatenate([s0[0:128], s0[128:256]], axis=1).astype(bf)
        m_i["ut"] = np.concatenate(
            [utt[k * 128:(k + 1) * 128] for k in range(4)], axis=1
        ).astype(bf)
        in_maps.append(m_i)
    return in_maps


def kernel(y, z, u, mu, sigma_inv, a_coef, b_coef, bias, _trace=False):
    if "nc" not in _CACHE:
        _CACHE["nc"] = build_program()
    nc = _CACHE["nc"]
    in_maps = host_prep(y, z, u, mu, sigma_inv, a_coef, b_coef, bias)
    res = run_bass_kernel_spmd(
        nc, in_maps, core_ids=list(range(N_CORES)), trace=_trace
    )
    _CACHE["last_result"] = res
    out = np.concatenate(
        [res.results[i]["outT"].T[:, None, :] for i in range(N_CORES)], axis=0
    )
    return out


# revision 42
# speedup vs baseline: 1.2978x; 1.2978x over previous
"""Trainium2 Bass kernel for nn_EvolvingSystem (moe_routing).

Math (reference):
  psi = softmax_c(-d2),  d2[b,c] = (mu_c - z_b)^T S_c (mu_c - z_b),  S_c = si_c si_c^T
  ARX: preds[b,c,l] from linear recursion on state0 = y[:,:,-16:] and
       ub[b,c] = u[b,c,:].b_coef[c] + bias[c]
  out[b,l] = sum_c psi[b,c] preds[b,c,l]

Device strategy (8 cores, data-parallel on B, 1024 rows/core):
  d2[b,c] = ||t_bc||^2 - 2 z_b.q_c + k_c   with t_bc = si_c^T z_b
    -> big matmul T = Z @ si_c per cluster pair (fp32r, full PE rate),
       z^T tiles stationary (lhsT), sigma streams (rhs, N=512).
  Cluster columns are INTERLEAVED in each [128,512] tile (even/odd lanes =
  the two clusters). Sum-of-squares drains split per (pair,bk) tile across
  the two PSUM-capable vector engines (GPSIMD/Pool cannot touch PSUM, and
  DVE may read only one PSUM operand, which rules out tensor_tensor_reduce):
    T-tiles (44): DVE bn_stats (sumsq = M2 + 256*mean^2 fixup per bk)
    A-tiles (20): ACT Square+accum_out per cluster on strided lanes
  The per-bk T-pair sets are kept to all/evens/odds so the fixup writes are
  regular rearrange views. Softmax keeps the min-subtraction pass: tr(S) is
  ~90 so d2 reaches ~130 and raw exp(-d2) underflows f32 for ~12% of rows.
  ARX recursion is linear -> host-unrolled coefficients W[c,l,o], g[c,l]:
    preds[b,c,l] = sum_o W[c,l,o] state0[b,c,o] + g[c,l] ub[b,c]
    out^T[l,b] = Wflat^T @ (psi*state0)^T + g^T @ (psi*ub)^T   (small matmuls)
  u/state0/bmat ship as bf16 (error contribution measured ~0); z/sigma must
  stay fp32 (bf16 sigma alone puts final rel-err over the 2e-2 gate).

DMA: few, large transfers. sync(SP) queue: z khalf0 + 8 sigma slabs
[128,1024] + s0t + ut; scalar(ACT) queue: z khalf1 + packed params (its
activation work starts only after pair0 lands). Pool only computes.
Softmax for batch-chunk bk is emitted right after the last pair's drain of
that bk so it overlaps the remaining main matmuls.
"""

import sys
from contextlib import ExitStack

import numpy as np

if "/opt/trn_rl_repo" not in sys.path:
    sys.path.insert(0, "/opt/trn_rl_repo")

import ml_dtypes

import concourse.bass as bass
import concourse.mybir as mybir
import concourse.tile as tile
from concourse import bacc
from concourse.bass_utils import run_bass_kernel_spmd

N_CORES = 8
B, C, D = 8192, 16, 256
R, E, ORD, L = 64, 32, 16, 32
BLOC = B // N_CORES            # 1024
NBK = BLOC // 128              # 8 batch chunks of 128
CE = C * E                     # 512
CO = C * ORD                   # 256
NPAIR = C // 2                 # 8 cluster pairs
NSLAB = NPAIR // 2             # 4 sigma DMA slabs (2 pairs each)

F32 = mybir.dt.float32
F32R = mybir.dt.float32r
BF16 = mybir.dt.bfloat16
F16 = mybir.dt.float16

# per-(pair,bk) drain route: T=DVE bn_stats, A=ACT square+accum.
# 44 T / 20 A balances DVE vs ACT busy time; each bk's T-pair set is
# all/evens/odds so the fixup output views stay regular.
_ALLB, _EVENB, _ODDB = {1, 4, 6}, {0, 3, 5}, {2, 7}


def _is_t(pair, bk):
    return (
        bk in _ALLB
        or (bk in _EVENB and pair % 2 == 0)
        or (bk in _ODDB and pair % 2 == 1)
    )


TSET = [
    tuple(p for p in range(NPAIR) if _is_t(p, bk)) for bk in range(NBK)
]

_CACHE = {}


def build_program():
    nc = bacc.Bacc(
        "TRN2",
        target_bir_lowering=False,
        debug=False,
        enable_asserts=False,
        num_devices=N_CORES,
    )

    # ---- DRAM I/O (per-core shapes) ----
    zta_d = nc.dram_tensor("zta", [128, BLOC], F32R, kind="ExternalInput").ap()
    ztb_d = nc.dram_tensor("ztb", [128, BLOC], F32R, kind="ExternalInput").ap()
    # qa[i, 2*half+c] = -2 q[c, half*128+i];  kfill[i, bk*16+c] = k_c
    qa_d = nc.dram_tensor("qa", [128, 2 * C], F32R, kind="ExternalInput").ap()
    krow_d = nc.dram_tensor("krow", [1, 128], F32R, kind="ExternalInput").ap()
    # sgr[h, i, c*256 + j] = sigma_inv[c, h*128+i, j]  (cluster-major cols)
    sgr_d = nc.dram_tensor("sgr", [2, 128, NPAIR * 512], F32R, kind="ExternalInput").ap()
    s0t_d = nc.dram_tensor("s0t", [128, 2 * BLOC], BF16, kind="ExternalInput").ap()
    ut_d = nc.dram_tensor("ut", [128, 4 * BLOC], BF16, kind="ExternalInput").ap()
    # pk16 = [emat | gmat | biasv] on 16 partitions
    pk16_d = nc.dram_tensor("pk16", [C, CO + L + 1], F32R, kind="ExternalInput").ap()
    wfp_d = nc.dram_tensor("wfp", [128, 2 * L], F32R, kind="ExternalInput").ap()
    bmp_d = nc.dram_tensor("bmp", [128, 4 * C], BF16, kind="ExternalInput").ap()
    ident_d = nc.dram_tensor("ident", [128, 128], F32, kind="ExternalInput").ap()
    out_d = nc.dram_tensor("outT", [L, BLOC], F32, kind="ExternalOutput").ap()

    with tile.TileContext(nc) as tc, ExitStack() as ctx:
        const = ctx.enter_context(tc.tile_pool(name="const", bufs=1))
        scr_a = ctx.enter_context(tc.tile_pool(name="scr_a", bufs=3))
        sqp = ctx.enter_context(tc.tile_pool(name="sqp", bufs=NBK))
        stp = ctx.enter_context(tc.tile_pool(name="stp", bufs=NBK))
        soft = ctx.enter_context(tc.tile_pool(name="soft", bufs=4))
        tailp = ctx.enter_context(tc.tile_pool(name="tailp", bufs=4))
        ps_t = ctx.enter_context(tc.tile_pool(name="ps_t", bufs=5, space="PSUM"))
        ps_dots = ctx.enter_context(tc.tile_pool(name="ps_dots", bufs=1, space="PSUM"))
        ps_tail = ctx.enter_context(tc.tile_pool(name="ps_tail", bufs=2, space="PSUM"))

        # ---- startup loads ----
        # A single DMA's transfer streams at ~41 B/ns (~2 of the 16 rings)
        # with ~3 in flight per queue, so large tensors are CHUNKED and
        # issued in need-order across the queues. s0t/ut trail the sigma
        # stream so they don't steal bandwidth from the critical path.
        zta = const.tile([128, BLOC], F32R, tag="zta", name="zta")
        ztb = const.tile([128, BLOC], F32R, tag="ztb", name="ztb")
        sg = [
            [const.tile([128, 512], F32R, tag=f"sg{p}{h}", name=f"sg{p}{h}")
             for h in range(2)]
            for p in range(NPAIR)
        ]
        qa = const.tile([128, 2 * C], F32R, tag="qa", name="qa")
        dots = ps_dots.tile([128, 128], F32, tag="dots", name="dots")

        def zc(i):
            return slice(i * 256, (i + 1) * 256)

        # sync: zta chunks + sigma khalf0 + tail tensors
        nc.sync.dma_start(zta[:, zc(0)], zta_d[:, zc(0)])
        nc.sync.dma_start(sg[0][0][:], sgr_d[0, :, 0:512])
        for i in range(1, 4):
            nc.sync.dma_start(zta[:, zc(i)], zta_d[:, zc(i)])
        for p in range(1, NPAIR):
            nc.sync.dma_start(sg[p][0][:], sgr_d[0, :, p * 512:(p + 1) * 512])
        s0t = const.tile([128, 2 * BLOC], BF16, tag="s0t", name="s0t")
        ut = const.tile([128, 4 * BLOC], BF16, tag="ut", name="ut")
        for i in range(2):
            cs = slice(i * BLOC, (i + 1) * BLOC)
            nc.sync.dma_start(s0t[:, cs], s0t_d[:, cs])
        for i in range(4):
            cs = slice(i * BLOC, (i + 1) * BLOC)
            nc.sync.dma_start(ut[:, cs], ut_d[:, cs])

        # scalar: only the earliest-needed smalls (its ACT compute starts ~13us)
        nc.scalar.dma_start(qa[:], qa_d[:])
        nc.scalar.dma_start(ztb[:, zc(0)], ztb_d[:, zc(0)])
        krow = const.tile([1, 128], F32R, tag="krow", name="krow")
        nc.scalar.dma_start(krow[:], krow_d[:])

        # gpsimd (otherwise idle): sigma khalf1 stream + rest via SWDGE
        nc.gpsimd.dma_start(sg[0][1][:], sgr_d[1, :, 0:512])
        for i in range(1, 4):
            nc.gpsimd.dma_start(ztb[:, zc(i)], ztb_d[:, zc(i)])
        for p in range(1, NPAIR):
            nc.gpsimd.dma_start(sg[p][1][:], sgr_d[1, :, p * 512:(p + 1) * 512])
        pk16 = const.tile([C, CO + L + 1], F32R, tag="pk16", name="pk16")
        nc.gpsimd.dma_start(pk16[:], pk16_d[:])
        wfp = const.tile([128, 2 * L], F32R, tag="wfp", name="wfp")
        nc.gpsimd.dma_start(wfp[:], wfp_d[:])
        bmp = const.tile([128, 4 * C], BF16, tag="bmp", name="bmp")
        nc.gpsimd.dma_start(bmp[:], bmp_d[:])
        ident = const.tile([128, 128], F32, tag="ident", name="ident")
        nc.gpsimd.dma_start(ident[:], ident_d[:])

        emat = pk16[:, 0:CO]
        gmat = pk16[:, CO:CO + L]
        biasv = pk16[:, CO + L:CO + L + 1]

        # ---- dots[b, bk*16+c] = -2 z.q + k  (one PSUM bank, col-sliced) ----
        # One rank-1 matmul broadcasts k into the whole bank (ones^T @ krow),
        # then the per-bk slices accumulate on top - no per-bk ones matmuls.
        ones = const.tile([1, 128], F32R, tag="ones", name="ones")
        nc.gpsimd.memset(ones[:].bitcast(mybir.dt.uint32), 0x3F800000)
        nc.tensor.matmul(dots[:], ones[:], krow[:], start=True, stop=False,
                         skip_group_check=True)
        for bk in range(NBK):
            sl = dots[:, bk * C:(bk + 1) * C]
            bsl = slice(bk * 128, (bk + 1) * 128)
            nc.tensor.matmul(sl, zta[:, bsl], qa[:, 0:C], start=False,
                             stop=False, skip_group_check=True)
            nc.tensor.matmul(sl, ztb[:, bsl], qa[:, C:2 * C], start=False,
                             stop=True, skip_group_check=True)

        sqacc = [sqp.tile([128, C], F32, tag="sqacc", name="sqacc") for _ in range(NBK)]
        stats = [
            stp.tile([128, NPAIR, 6], F32, tag="stats", name="stats")
            for _ in range(NBK)
        ]
        psit_r = const.tile([C, BLOC], F32R, tag="psit_r", name="psit_r")
        psi4 = [
            const.tile([128, 128], F32, tag=f"psi4{g}", name=f"psi4{g}")
            for g in range(2)
        ]
        for g in range(2):
            # pad lanes are transposed and then ignored; zero them so the
            # simulator never sees uninitialized reads
            nc.gpsimd.memset(psi4[g][:].bitcast(mybir.dt.uint32), 0)

        def drain(pair, bk, t_ps, eng):
            if eng == "T":
                slot = TSET[bk].index(pair)
                nc.vector.bn_stats(stats[bk][:, slot, :], t_ps[:])
            else:
                for cc in range(2):
                    acc = sqacc[bk][:, 2 * pair + cc:2 * pair + cc + 1]
                    o = scr_a.tile([128, 256], F32, tag="scra", name="scra")
                    nc.scalar.activation(
                        o[:], t_ps[:, cc::2],
                        mybir.ActivationFunctionType.Square,
                        accum_out=acc,
                    )

        def fixup_bk(bk):
            # DVE tiles: sumsq = M2 + 256*mean^2 per cluster (even/odd stats)
            ts = TSET[bk]
            if not ts:
                return
            nt = len(ts)
            st = stats[bk]
            v_mu = st[:, 0:nt, 1:6:3]    # [128, nt, 2] means (even, odd)
            v_m2 = st[:, 0:nt, 2:6:3]    # [128, nt, 2] M2 = count*var
            if nt == NPAIR:
                o = sqacc[bk][:, 0:16].rearrange("p (g x) -> p g x", x=2)
            elif ts[0] == 0:             # even pairs -> cols (4g, 4g+1)
                o = sqacc[bk][:, 0:16].rearrange("p (g x) -> p g x", x=4)[:, :, 0:2]
            else:                        # odd pairs -> cols (4g+2, 4g+3)
                o = sqacc[bk][:, 0:16].rearrange("p (g x) -> p g x", x=4)[:, :, 2:4]
            tmp = soft.tile([128, nt, 2], F32, tag="fix", name="fix")
            nc.vector.tensor_tensor(tmp[:], v_mu, v_mu, op=mybir.AluOpType.mult)
            nc.vector.scalar_tensor_tensor(
                out=o, in0=tmp[:], scalar=256.0, in1=v_m2,
                op0=mybir.AluOpType.mult, op1=mybir.AluOpType.add,
            )

        def softmax_bk(bk):
            fixup_bk(bk)
            d2 = soft.tile([128, C], F32, tag="d2", name="d2")
            nc.vector.scalar_tensor_tensor(
                out=d2[:], in0=dots[:, bk * C:(bk + 1) * C], scalar=1.0,
                in1=sqacc[bk][:], op0=mybir.AluOpType.mult,
                op1=mybir.AluOpType.add,
            )
            dmin = soft.tile([128, 1], F32, tag="dmin", name="dmin")
            nc.vector.tensor_reduce(
                dmin[:], d2[:], axis=mybir.AxisListType.X, op=mybir.AluOpType.min
            )
            et = soft.tile([128, C], F32, tag="et", name="et")
            den = soft.tile([128, 1], F32, tag="den", name="den")
            nc.scalar.activation(
                et[:], d2[:], mybir.ActivationFunctionType.Exp,
                bias=dmin[:], scale=-1.0, accum_out=den[:],
            )
            rden = soft.tile([128, 1], F32, tag="rden", name="rden")
            nc.vector.reciprocal(rden[:], den[:])
            g = bk // 4
            # each chunk's psi sits at a 32-col boundary so the post-transpose
            # partition offsets (32j) are engine-legal
            nc.vector.tensor_scalar_mul(
                psi4[g][:, (bk % 4) * 32:(bk % 4) * 32 + C], et[:], rden[:]
            )
            if bk % 4 == 3:
                # one PE transpose per 4 batch chunks, then per-chunk copies
                pt_ps = ps_tail.tile([128, 128], F32, tag="tail", name="tail")
                nc.tensor.transpose(pt_ps[:], psi4[g][:], ident[:])
                for j in range(4):
                    nc.scalar.activation(
                        psit_r[:, (4 * g + j) * 128:(4 * g + j + 1) * 128],
                        pt_ps[j * 32:j * 32 + C, :],
                        mybir.ActivationFunctionType.Copy,
                    )

        # ---- main: T = Z @ si per (pair, bk); DVE/ACT drains ----
        # Phase 1 (sigma still streaming in): pairs 0-3, bk-inner.
        # Phase 2 (all sigma resident): bk-outer over pairs 4-7, with each
        # bk's softmax fused right after its last pair so the whole softmax+
        # transpose chain overlaps the remaining matmuls instead of
        # serializing after them.
        def main_tile(pair, bk):
            bsl = slice(bk * 128, (bk + 1) * 128)
            t_ps = ps_t.tile([128, 512], F32, tag="t_ps", name="t_ps")
            nc.tensor.matmul(t_ps[:], zta[:, bsl], sg[pair][0][:],
                             start=True, stop=False)
            nc.tensor.matmul(t_ps[:], ztb[:, bsl], sg[pair][1][:],
                             start=False, stop=True)
            drain(pair, bk, t_ps, "T" if _is_t(pair, bk) else "A")

        for pair in range(4):
            for bk in range(NBK):
                main_tile(pair, bk)
        for bk in range(NBK):
            for pair in range(4, NPAIR):
                main_tile(pair, bk)
            softmax_bk(bk)

        # ---- tail (all in [*, b] orientation, b512 chunks) ----
        for bh in range(2):
            bsl = slice(bh * 512, (bh + 1) * 512)
            psie = []
            for k in range(2):
                p = ps_tail.tile([128, 512], F32, tag="tail", name="tail")
                nc.tensor.matmul(
                    p[:], emat[:, k * 128:(k + 1) * 128], psit_r[:, bsl],
                    start=True, stop=True,
                )
                psie.append(p)
            a_sb = []
            for k in range(2):
                t = tailp.tile([128, 512], F32R, tag="a_sb", name="a_sb")
                nc.vector.tensor_tensor(
                    t[:], s0t[:, k * BLOC + bh * 512:k * BLOC + bh * 512 + 512],
                    psie[k][:], op=mybir.AluOpType.mult,
                )
                a_sb.append(t)
            ubp = ps_tail.tile([C, 512], F32, tag="tail", name="tail")
            for k in range(4):
                nc.tensor.matmul(
                    ubp[:], bmp[:, k * C:(k + 1) * C],
                    ut[:, k * BLOC + bh * 512:k * BLOC + bh * 512 + 512],
                    start=(k == 0), stop=(k == 3),
                )
            pt_sb = tailp.tile([C, 512], F32R, tag="pt_sb", name="pt_sb")
            nc.vector.scalar_tensor_tensor(
                out=pt_sb[:], in0=ubp[:], scalar=biasv, in1=psit_r[:, bsl],
                op0=mybir.AluOpType.add, op1=mybir.AluOpType.mult,
            )
            outp = ps_tail.tile([L, 512], F32, tag="tail", name="tail")
            nc.tensor.matmul(outp[:], wfp[:, 0:L], a_sb[0][:], start=True, stop=False)
            nc.tensor.matmul(outp[:], wfp[:, L:2 * L], a_sb[1][:], start=False, stop=False)
            nc.tensor.matmul(outp[:], gmat, pt_sb[:], start=False, stop=True)
            out_sb = tailp.tile([L, 512], F32, tag="out_sb", name="out_sb")
            nc.vector.tensor_copy(out_sb[:], outp[:])
            nc.sync.dma_start(out_d[:, bsl], out_sb[:])

    nc.compile()
    return nc


def host_prep(y, z, u, mu, sigma_inv, a_coef, b_coef, bias):
    """Host-side precompute: shared tensors + per-core input maps."""
    f64 = np.float64
    bf = ml_dtypes.bfloat16
    W = np.zeros((C, L, ORD), f64)
    g = np.zeros((C, L), f64)
    for c in range(C):
        a = a_coef[c].astype(f64)
        S = np.eye(ORD, dtype=f64)
        sb = np.zeros(ORD, f64)
        for l in range(L):
            ya = a @ S
            yb = a @ sb + 1.0
            W[c, l] = ya
            g[c, l] = yb
            S = np.vstack([S[1:], ya[None]])
            sb = np.concatenate([sb[1:], [yb]])
    wflat = np.ascontiguousarray(W.transpose(0, 2, 1).reshape(CO, L)).astype(np.float32)
    wfp = np.concatenate([wflat[0:128], wflat[128:256]], axis=1)
    gmat = g.astype(np.float32)

    si = sigma_inv.astype(f64)
    m = np.einsum("cij,ci->cj", si, mu.astype(f64))
    q = np.einsum("cij,cj->ci", si, m)          # S_c mu_c
    k = np.sum(m * m, axis=1)
    qt = (-2.0 * q.T).astype(np.float32)        # [D, C]
    qa = np.concatenate([qt[0:128], qt[128:256]], axis=1)   # [128, 2C]
    krow = np.tile(k.astype(np.float32), NBK).reshape(1, 128)

    # interleave each pair's two clusters in the column lanes (even/odd)
    sit = sigma_inv.transpose(1, 0, 2)          # [i, c, j]
    sgr = np.ascontiguousarray(
        sit.reshape(D, NPAIR, 2, D).transpose(0, 1, 3, 2).reshape(2, 128, NPAIR * 512)
    ).astype(np.float32)

    emat = np.zeros((C, CO), np.float32)
    for c in range(C):
        emat[c, c * ORD:(c + 1) * ORD] = 1.0
    pk16 = np.concatenate(
        [emat, gmat, bias.astype(np.float32).reshape(C, 1)], axis=1
    )

    bmat = np.zeros((CE, C), np.float32)
    for c in range(C):
        bmat[c * E:(c + 1) * E, c] = b_coef[c]
    bmp = np.concatenate([bmat[k * 128:(k + 1) * 128] for k in range(4)], axis=1)

    shared = {
        "qa": qa,
        "krow": krow,
        "sgr": sgr,
        "pk16": pk16,
        "wfp": wfp,
        "bmp": bmp.astype(bf),
        "ident": np.eye(128, dtype=np.float32),
    }
    in_maps = []
    for i in range(N_CORES):
        s = slice(i * BLOC, (i + 1) * BLOC)
        zt = np.ascontiguousarray(z[s, 0, :].T)             # [256, BLOC]
        s0 = np.ascontiguousarray(y[s, :, R - ORD:].reshape(BLOC, CO).T)
        utt = np.ascontiguousarray(u[s].reshape(BLOC, CE).T)
        m_i = dict(shared)
        m_i["zta"] = zt[0:128]
        m_i["ztb"] = zt[128:256]
        m_i["s0t"] = np.concatenate([s0[0:128], s0[128:256]], axis=1).astype(bf)
        m_i["ut"] = np.concatenate(
            [utt[k * 128:(k + 1) * 128] for k in range(4)], axis=1
        ).astype(bf)
        in_maps.append(m_i)
    return in_maps


def kernel(y, z, u, mu, sigma_inv, a_coef, b_coef, bias, _trace=False):
    if "nc" not in _CACHE:
        _CACHE["nc"] = build_program()
    nc = _CACHE["nc"]
    in_maps = host_prep(y, z, u, mu, sigma_inv, a_coef, b_coef, bias)
    res = run_bass_kernel_spmd(
        nc, in_maps, core_ids=list(range(N_CORES)), trace=_trace
    )
    _CACHE["last_result"] = res
    out = np.concatenate(
        [res.results[i]["outT"].T[:, None, :] for i in range(N_CORES)], axis=0
    )
    return out
